# revision 18
# baseline (speedup 1.0000x reference)
"""Distance-aware multihead attention on 8 Trainium2 cores.

Strategy
--------
rel = scatter(edge_attr) is ~98% sparse (49k edges into 1536^2 cells), so the
positional-encoding terms decompose into a rank-1 baseline (rel==0 rows of the
sinusoidal PE: [0,1,0,1,...]) plus sparse per-edge corrections:

  logits/sqrt(dh) = q~ . k~  +  0.125 * (q+k) . d_pe   (second term only at edges)
  q~ = [q, 0.125*qodd, 1],  k~ = [0.25*k, 1, 0.125*kodd]   (66-dim augmented)
  d_pe[e] = [sin(v_e*div_i), cos(v_e*div_i)-1] interleaved

Each edge contributes to softmax numerator/denominator through
  w[h,e] = exp(qk0+delta) - exp(qk0)
which is accumulated straight into the attention*V PSUM via a one-hot matmul
(lhsT = onehot(q_e), rhs = w * vhat_rows).  No dense [N,N,dh] tensor, no
scatter.  Logits are computed transposed ([r,q]) so the softmax denominator
and A@V come from one matmul with a ones-column appended to v (vhat), and the
output lands in [d,q] layout for the final max-over-nodes pooling.

Sharding: cores own 192 query rows each (sequence-parallel); k/v projections
are replicated (cheaper than collectives at this size); all 8 heads per core.
"""

import math

import numpy as np

import concourse.bass as bass
import concourse.mybir as mybir
import concourse.tile as tile
from concourse import bacc
from concourse.bass_utils import run_bass_kernel_spmd
from concourse.masks import make_identity

N, E, D, H, DH = 1536, 49152, 512, 8, 64
NC = 8
QC = N // NC          # 192 query rows per core
P = 128
SCALE = 256.0 * math.sqrt(2.0)
FP = mybir.dt.float32
I32 = mybir.dt.int32
VW = H * 65           # 520: vhat/table row width (per-head 64 + 1 aux col)
BT = 4                # edge tiles per compute batch

AluOp = mybir.AluOpType
ActFn = mybir.ActivationFunctionType


# ----------------------------------------------------------------- host prep
def _prep_edges(edge_index, edge_attr):
    ei = np.asarray(edge_index)
    ea = np.asarray(edge_attr, np.float64)
    flat = ei[0].astype(np.int64) * N + ei[1].astype(np.int64)
    uniq, inv = np.unique(flat, return_inverse=True)
    val = np.zeros(len(uniq), np.float64)
    np.add.at(val, inv, ea)
    val *= SCALE
    qe = (uniq // N).astype(np.int64)
    re_ = (uniq % N).astype(np.int64)
    div = np.exp(np.arange(0, DH, 2, dtype=np.float64) * (-math.log(10000.0) / DH))
    ang = val[:, None] * div[None, :]
    pev = np.empty((len(uniq), DH), np.float32)
    pev[:, 0::2] = 0.125 * np.sin(ang)
    pev[:, 1::2] = 0.125 * (np.cos(ang) - 1.0)
    return qe, re_, pev


def _shard_edges(qe, re_, pev):
    """Per-core edge arrays: grouped by q-block (q<128 | q>=128), padded so
    every core has the same tile counts (T0, T1), both multiples of BT."""
    percore = []
    for c in range(NC):
        sel = (qe >= c * QC) & (qe < (c + 1) * QC)
        ql = (qe[sel] - c * QC).astype(np.int32)
        rr = re_[sel].astype(np.int32)
        pv = pev[sel]
        g0 = ql < P
        percore.append((ql, rr, pv, g0))

    def ntiles(count):
        return -(-max(count, 1) // P)

    T0 = max(ntiles(int(g0.sum())) for (_, _, _, g0) in percore)
    T1 = max(ntiles(int((~g0).sum())) for (_, _, _, g0) in percore)
    T0 = -(-T0 // BT) * BT
    T1 = -(-T1 // BT) * BT

    maps = []
    for ql, rr, pv, g0 in percore:
        qli = np.zeros(P * (T0 + T1), np.int32)
        rei = np.zeros(P * (T0 + T1), np.int32)
        qlf = np.zeros(P * (T0 + T1), np.float32)
        pvv = np.zeros((P * (T0 + T1), DH), np.float32)
        n0 = int(g0.sum())
        qli[:n0] = ql[g0]
        rei[:n0] = rr[g0]
        qlf[:n0] = ql[g0].astype(np.float32)
        pvv[:n0] = pv[g0]
        # padding rows of group 0 keep ql=0 (valid), pev=0 -> w=0 -> no-op
        o = P * T0
        n1 = int((~g0).sum())
        qli[o:o + n1] = ql[~g0]
        qli[o + n1:] = P  # padding for block 1 must stay in-block
        rei[o:o + n1] = rr[~g0]
        qlf[o:o + n1] = (ql[~g0] - P).astype(np.float32)
        pvv[o:o + n1] = pv[~g0]
        maps.append(dict(qli=qli, rei=rei, qlf=qlf, pev=pvv))
    return T0, T1, maps


# -------------------------------------------------------------- device build
def _build(T0, T1, dbg=False):
    nc = bacc.Bacc("TRN2", target_bir_lowering=False, debug=False,
                   enable_asserts=False)
    EP = P * (T0 + T1)

    featT_d = nc.dram_tensor("featT", [D, N], FP, kind="ExternalInput").ap()
    qfeatT_d = nc.dram_tensor("qfeatT", [D, QC], FP, kind="ExternalInput").ap()
    wqT_d = nc.dram_tensor("wqT", [D, D], FP, kind="ExternalInput").ap()
    wkT_d = nc.dram_tensor("wkT", [D, D], FP, kind="ExternalInput").ap()
    wvT_d = nc.dram_tensor("wvT", [D, D], FP, kind="ExternalInput").ap()
    modd_d = nc.dram_tensor("modd", [D, H], FP, kind="ExternalInput").ap()
    qli_d = nc.dram_tensor("qli", [EP], I32, kind="ExternalInput").ap()
    rei_d = nc.dram_tensor("rei", [EP], I32, kind="ExternalInput").ap()
    qlf_d = nc.dram_tensor("qlf", [EP], FP, kind="ExternalInput").ap()
    pev_d = nc.dram_tensor("pev", [EP, DH], FP, kind="ExternalInput").ap()
    out_d = nc.dram_tensor("out_part", [D], FP, kind="ExternalOutput").ap()

    rec_d = nc.dram_tensor("recd", [H, QC], FP, kind="Internal").ap()
    koddT_d = nc.dram_tensor("koddTd", [H, N], FP, kind="Internal").ap()
    qoddT_d = nc.dram_tensor("qoddTd", [H, QC], FP, kind="Internal").ap()
    if dbg:
        EPt = P * (T0 + T1)
        dbg_w = nc.dram_tensor("dbgw", [EPt // P, P, H], FP, kind="Internal").ap()
        dbg_dl = nc.dram_tensor("dbgdl", [EPt // P, P, H], FP, kind="Internal").ap()
        dbg_oh = nc.dram_tensor("dbgoh", [EPt // P, P, P], FP, kind="Internal").ap()
        dbg_corr = nc.dram_tensor("dbgcorr", [2, P, VW], FP, kind="Internal").ap()
        dbg_av = nc.dram_tensor("dbgav", [H, 65, QC], FP, kind="Internal").ap()
    qrowx_d = nc.dram_tensor("qrowx", [QC, VW], FP, kind="Internal").ap()
    krowx_d = nc.dram_tensor("krowx", [N, VW], FP, kind="Internal").ap()
    vhat_d = nc.dram_tensor("vhatd", [N, VW], FP, kind="Internal").ap()

    NB = N // P       # 12 node blocks
    KC = D // P       # 4 contraction chunks
    QB = [(0, P), (P, QC - P)]   # q blocks: (offset, size)

    from contextlib import ExitStack
    with tile.TileContext(nc) as tc, ExitStack() as stk:
        cpool = stk.enter_context(tc.tile_pool(name="const", bufs=1))
        ident = cpool.tile([P, P], FP)
        make_identity(nc, ident[:])
        iota = cpool.tile([P, P], FP)
        nc.gpsimd.iota(iota[:], pattern=[[1, P]], base=0, channel_multiplier=0,
                       allow_small_or_imprecise_dtypes=True)

        # ---- persistent SBUF (live through dense phase) ----
        big = stk.enter_context(tc.tile_pool(name="big", bufs=1))
        # augmented-K layout (SBUF base-partition must be 0/32/64/96):
        #   khatT rows: 0-63 = 0.25*k_h, 64 = ones, 96 = 0.125*kodd_h, rest 0
        #   qhatT rows: 0-63 = q_h, 64 = 0.125*qodd_h, 96 = ones, rest 0
        vhat = big.tile([P, NB * VW], FP)          # [r%128, (rblk)(h*65+c)]
        khatT = big.tile([P, H * N], FP)           # [128, (h)(r)]
        qhatT = big.tile([P, H * QC], FP)          # [128, (h)(q)]
        corr0 = big.tile([P, VW], FP)              # corr AV, q block 0
        corr1 = big.tile([QB[1][1], VW], FP)       # corr AV, q block 1

        # ================= phase 1: projections ==================
        with tc.tile_pool(name="pin", bufs=1) as pin, \
             tc.tile_pool(name="stage", bufs=4) as stage, \
             tc.tile_pool(name="prj", bufs=2, space="PSUM") as prj, \
             tc.tile_pool(name="sps", bufs=1, space="PSUM") as sps, \
             tc.tile_pool(name="mid", bufs=1) as mid:

            featT = pin.tile([P, KC * N], FP)
            wqT = pin.tile([P, KC * D], FP)
            wkT = pin.tile([P, KC * D], FP)
            wvT = pin.tile([P, KC * D], FP)
            qfeatT = pin.tile([P, KC * QC], FP)
            modd = pin.tile([P, KC * H], FP)
            for kc in range(KC):
                r = slice(kc * P, (kc + 1) * P)
                nc.sync.dma_start(featT[:, kc * N:(kc + 1) * N], featT_d[r, :])
                nc.sync.dma_start(wqT[:, kc * D:(kc + 1) * D], wqT_d[r, :])
                nc.sync.dma_start(wkT[:, kc * D:(kc + 1) * D], wkT_d[r, :])
                nc.sync.dma_start(wvT[:, kc * D:(kc + 1) * D], wvT_d[r, :])
                nc.sync.dma_start(qfeatT[:, kc * QC:(kc + 1) * QC], qfeatT_d[r, :])
                nc.sync.dma_start(modd[:, kc * H:(kc + 1) * H], modd_d[r, :])

            kT = mid.tile([P, KC * N], FP)         # 0.25 * k, [dq, n]
            qT = mid.tile([P, KC * QC], FP)        # raw q, [dq, q]
            koddT = mid.tile([H, N], FP)           # 0.5 * (0.25k)odd = 0.125*kodd
            qoddT = mid.tile([H, QC], FP)          # 0.125 * qodd

            # kT (scaled 0.25) and qT, [dq, n] layout
            for dqc in range(KC):
                for nb3 in range(3):
                    ps = prj.tile([P, 512], FP, space="PSUM", tag="prjps")
                    for kc in range(KC):
                        nc.tensor.matmul(
                            out=ps[:],
                            lhsT=wkT[:, kc * D + dqc * P: kc * D + (dqc + 1) * P],
                            rhs=featT[:, kc * N + nb3 * 512: kc * N + (nb3 + 1) * 512],
                            start=(kc == 0), stop=(kc == KC - 1))
                    nc.vector.tensor_scalar_mul(
                        kT[:, dqc * N + nb3 * 512: dqc * N + (nb3 + 1) * 512],
                        ps[:], 0.25)
                ps = prj.tile([P, 512], FP, space="PSUM", tag="prjps")
                for kc in range(KC):
                    nc.tensor.matmul(
                        out=ps[:, :QC],
                        lhsT=wqT[:, kc * D + dqc * P: kc * D + (dqc + 1) * P],
                        rhs=qfeatT[:, kc * QC:(kc + 1) * QC],
                        start=(kc == 0), stop=(kc == KC - 1))
                nc.vector.tensor_copy(qT[:, dqc * QC:(dqc + 1) * QC], ps[:, :QC])

            # koddT [8, N] = 0.5 * modd^T kT ; qoddT [8, QC] = 0.125 * modd^T qT
            for nb3 in range(3):
                ps = sps.tile([H, 512], FP, space="PSUM", tag="oddT")
                for dqc in range(KC):
                    nc.tensor.matmul(
                        out=ps[:],
                        lhsT=modd[:, dqc * H:(dqc + 1) * H],
                        rhs=kT[:, dqc * N + nb3 * 512: dqc * N + (nb3 + 1) * 512],
                        start=(dqc == 0), stop=(dqc == KC - 1))
                nc.vector.tensor_scalar_mul(
                    koddT[:, nb3 * 512:(nb3 + 1) * 512], ps[:], 0.5)
            ps = sps.tile([H, 512], FP, space="PSUM", tag="oddT")
            for dqc in range(KC):
                nc.tensor.matmul(
                    out=ps[:, :QC], lhsT=modd[:, dqc * H:(dqc + 1) * H],
                    rhs=qT[:, dqc * QC:(dqc + 1) * QC],
                    start=(dqc == 0), stop=(dqc == KC - 1))
            nc.vector.tensor_scalar_mul(qoddT[:], ps[:, :QC], 0.125)

            # khatT / qhatT assembly (odd rows bounce through DRAM so every
            # SBUF access starts at partition 0/32/64/96)
            nc.sync.dma_start(koddT_d[:, :], koddT[:])
            nc.sync.dma_start(qoddT_d[:, :], qoddT[:])
            nc.vector.memset(khatT[64:96, :], 0.0)
            nc.vector.memset(khatT[96:128, :], 0.0)
            nc.vector.memset(khatT[64:65, :], 1.0)
            nc.vector.memset(qhatT[64:96, :], 0.0)
            nc.vector.memset(qhatT[96:128, :], 0.0)
            nc.vector.memset(qhatT[96:97, :], 1.0)
            for h in range(H):
                dqc, half = divmod(h, 2)
                rows = slice(half * 64, half * 64 + 64)
                nc.sync.dma_start(khatT[0:64, h * N:(h + 1) * N],
                                  kT[rows, dqc * N:(dqc + 1) * N])
                nc.sync.dma_start(qhatT[0:64, h * QC:(h + 1) * QC],
                                  qT[rows, dqc * QC:(dqc + 1) * QC])
                nc.sync.dma_start(khatT[96:97, h * N:(h + 1) * N],
                                  koddT_d[h:h + 1, :])
                nc.sync.dma_start(qhatT[64:65, h * QC:(h + 1) * QC],
                                  qoddT_d[h:h + 1, :])

            # krow / vhat / qrow tables (row layouts) + odd columns -> DRAM
            for nb in range(NB):
                psk = prj.tile([P, 512], FP, space="PSUM", tag="rowk")
                psv = prj.tile([P, 512], FP, space="PSUM", tag="rowv")
                for kc in range(KC):
                    lhs = featT[:, kc * N + nb * P: kc * N + (nb + 1) * P]
                    nc.tensor.matmul(out=psk[:], lhsT=lhs,
                                     rhs=wkT[:, kc * D:(kc + 1) * D],
                                     start=(kc == 0), stop=(kc == KC - 1))
                    nc.tensor.matmul(out=psv[:], lhsT=lhs,
                                     rhs=wvT[:, kc * D:(kc + 1) * D],
                                     start=(kc == 0), stop=(kc == KC - 1))
                # kodd for these nodes: modd^T contraction via kT (x4 unscale)
                pso = sps.tile([P, H], FP, space="PSUM", tag="oddrow")
                for dqc in range(KC):
                    nc.tensor.matmul(
                        out=pso[:],
                        lhsT=kT[:, dqc * N + nb * P: dqc * N + (nb + 1) * P],
                        rhs=modd[:, dqc * H:(dqc + 1) * H],
                        start=(dqc == 0), stop=(dqc == KC - 1))
                krow_s = stage.tile([P, 512], FP, tag="krs")
                kodd_s = stage.tile([P, H], FP, tag="kos")
                nc.vector.tensor_copy(krow_s[:], psk[:])
                nc.vector.tensor_scalar_mul(kodd_s[:], pso[:], 4.0)
                nc.sync.dma_start(krowx_d[nb * P:(nb + 1) * P, 0:512], krow_s[:])
                nc.sync.dma_start(krowx_d[nb * P:(nb + 1) * P, 512:VW], kodd_s[:])
                # vhat: [h*65 : h*65+64] = v, col h*65+64 = 1.0
                vslab = vhat[:, nb * VW:(nb + 1) * VW]
                vv = vslab.rearrange("p (h c) -> p h c", c=65)
                nc.vector.tensor_copy(
                    vv[:, :, 0:64], psv[:].rearrange("p (h c) -> p h c", c=64))
                nc.vector.memset(vv[:, :, 64:65], 1.0)
                nc.sync.dma_start(vhat_d[nb * P:(nb + 1) * P, :], vslab)

            for qb, (qo, qn) in enumerate(QB):
                psq = prj.tile([P, 512], FP, space="PSUM", tag="rowk")
                for kc in range(KC):
                    nc.tensor.matmul(
                        out=psq[:qn, :],
                        lhsT=qfeatT[:, kc * QC + qo: kc * QC + qo + qn],
                        rhs=wqT[:, kc * D:(kc + 1) * D],
                        start=(kc == 0), stop=(kc == KC - 1))
                pso = sps.tile([P, H], FP, space="PSUM", tag="oddrow")
                for dqc in range(KC):
                    nc.tensor.matmul(
                        out=pso[:qn, :],
                        lhsT=qT[:, dqc * QC + qo: dqc * QC + qo + qn],
                        rhs=modd[:, dqc * H:(dqc + 1) * H],
                        start=(dqc == 0), stop=(dqc == KC - 1))
                qrow_s = stage.tile([P, 512], FP, tag="krs")
                qodd_s = stage.tile([P, H], FP, tag="kos")
                nc.vector.tensor_copy(qrow_s[:qn, :], psq[:qn, :])
                nc.vector.tensor_copy(qodd_s[:qn, :], pso[:qn, :])
                nc.sync.dma_start(qrowx_d[qo:qo + qn, 0:512], qrow_s[:qn, :])
                nc.sync.dma_start(qrowx_d[qo:qo + qn, 512:VW], qodd_s[:qn, :])

        # ================= phase 2: sparse edge corrections ==================
        with tc.tile_pool(name="gat", bufs=2) as gat, \
             tc.tile_pool(name="esc", bufs=2) as esc, \
             tc.tile_pool(name="corrps", bufs=1, space="PSUM") as corrps:

            for qb, (qo, qn) in enumerate(QB):
                t_lo = 0 if qb == 0 else T0
                t_hi = T0 if qb == 0 else T0 + T1
                cps_a = corrps.tile([qn, 4 * 65], FP, space="PSUM",
                                    tag=f"ca{qb}")
                cps_b = corrps.tile([qn, 4 * 65], FP, space="PSUM",
                                    tag=f"cb{qb}")
                for b0 in range(t_lo, t_hi, BT):
                    gq = gat.tile([P, BT * VW], FP, tag="gq")
                    gk = gat.tile([P, BT * VW], FP, tag="gk")
                    gv = gat.tile([P, BT * VW], FP, tag="gv")
                    qls = esc.tile([P, BT], I32, tag="qls")
                    res = esc.tile([P, BT], I32, tag="res")
                    qlfs = esc.tile([P, BT], FP, tag="qlfs")
                    pvs = esc.tile([P, BT * DH], FP, tag="pvs")
                    a, b = b0 * P, (b0 + BT) * P
                    nc.sync.dma_start(
                        qls[:], qli_d[a:b].rearrange("(t p) -> p t", p=P))
                    nc.sync.dma_start(
                        res[:], rei_d[a:b].rearrange("(t p) -> p t", p=P))
                    nc.sync.dma_start(
                        qlfs[:], qlf_d[a:b].rearrange("(t p) -> p t", p=P))
                    nc.sync.dma_start(
                        pvs[:].rearrange("p (t d) -> p t d", d=DH),
                        pev_d[a:b, :].rearrange("(t p) d -> p t d", p=P))
                    oh = esc.tile([P, BT * P], FP, tag="oh")
                    gs = esc.tile([P, BT * 512], FP, tag="gs")
                    mm = esc.tile([P, BT * 512], FP, tag="mm")
                    sm = esc.tile([P, BT * H * 6], FP, tag="sm")
                    # small-slab columns: dt | dl | od | qk0 | e0 | w
                    SS = BT * H
                    dt, dl = sm[:, 0:SS], sm[:, SS:2 * SS]
                    od, qk0 = sm[:, 2 * SS:3 * SS], sm[:, 3 * SS:4 * SS]
                    e0, w = sm[:, 4 * SS:5 * SS], sm[:, 5 * SS:6 * SS]
                    vgw = esc.tile([P, BT * VW], FP, tag="vgw")
                    for j in range(BT):
                        nc.gpsimd.indirect_dma_start(
                            out=gq[:, j * VW:(j + 1) * VW], out_offset=None,
                            in_=qrowx_d[:, :],
                            in_offset=bass.IndirectOffsetOnAxis(
                                ap=qls[:, j:j + 1], axis=0))
                        nc.gpsimd.indirect_dma_start(
                            out=gk[:, j * VW:(j + 1) * VW], out_offset=None,
                            in_=krowx_d[:, :],
                            in_offset=bass.IndirectOffsetOnAxis(
                                ap=res[:, j:j + 1], axis=0))
                        nc.gpsimd.indirect_dma_start(
                            out=gv[:, j * VW:(j + 1) * VW], out_offset=None,
                            in_=vhat_d[:, :],
                            in_offset=bass.IndirectOffsetOnAxis(
                                ap=res[:, j:j + 1], axis=0))
                        nc.vector.tensor_tensor(
                            out=oh[:, j * P:(j + 1) * P],
                            in0=qlfs[:, j:j + 1].to_broadcast([P, P]),
                            in1=iota[:], op=AluOp.is_equal)
                    g4q = gq[:].rearrange("p (t c) -> p t c", c=VW)
                    g4k = gk[:].rearrange("p (t c) -> p t c", c=VW)
                    gs4 = gs[:].rearrange("p (t c) -> p t c", c=512)
                    nc.vector.tensor_tensor(out=gs4, in0=g4q[:, :, 0:512],
                                            in1=g4k[:, :, 0:512], op=AluOp.add)
                    # delta = sum_d (gq+gk)[h*64+d] * pev[d]  (pev prescaled /8)
                    m4 = mm[:].rearrange("p (t h d) -> p t h d", h=H, d=DH)
                    nc.vector.tensor_tensor(
                        out=m4,
                        in0=gs[:].rearrange("p (t h d) -> p t h d", h=H, d=DH),
                        in1=pvs[:].rearrange("p (t d) -> p t d", d=DH)[:, :, None, :]
                            .to_broadcast([P, BT, H, DH]),
                        op=AluOp.mult)
                    nc.vector.reduce_sum(
                        out=dl.rearrange("p (t h) -> p t h", h=H), in_=m4,
                        axis=mybir.AxisListType.X)
                    # qk dot
                    nc.vector.tensor_tensor(
                        out=m4,
                        in0=g4q[:, :, 0:512].rearrange("p t (h d) -> p t h d", d=DH),
                        in1=g4k[:, :, 0:512].rearrange("p t (h d) -> p t h d", d=DH),
                        op=AluOp.mult)
                    nc.vector.reduce_sum(
                        out=dt.rearrange("p (t h) -> p t h", h=H), in_=m4,
                        axis=mybir.AxisListType.X)
                    nc.vector.tensor_tensor(
                        out=od.rearrange("p (t h) -> p t h", h=H),
                        in0=g4q[:, :, 512:VW], in1=g4k[:, :, 512:VW],
                        op=AluOp.add)
                    # qk0 = 0.25*(4*0.25*... ) -> dt is raw q.k ; qk0=(2dt+od)/8
                    nc.vector.tensor_scalar(out=qk0, in0=dt, scalar1=2.0,
                                            scalar2=None, op0=AluOp.mult)
                    nc.vector.tensor_tensor(out=qk0, in0=qk0, in1=od,
                                            op=AluOp.add)
                    nc.vector.tensor_scalar(out=qk0, in0=qk0, scalar1=0.125,
                                            scalar2=None, op0=AluOp.mult)
                    nc.scalar.activation(out=e0, in_=qk0, func=ActFn.Exp)
                    nc.vector.tensor_tensor(out=w, in0=qk0, in1=dl,
                                            op=AluOp.add)
                    nc.scalar.activation(out=w, in_=w, func=ActFn.Exp)
                    nc.vector.tensor_tensor(out=w, in0=w, in1=e0,
                                            op=AluOp.subtract)
                    # VgW = gathered vhat rows * w  (ones col -> w itself)
                    nc.vector.tensor_tensor(
                        out=vgw[:].rearrange("p (t h c) -> p t h c", h=H, c=65),
                        in0=gv[:].rearrange("p (t h c) -> p t h c", h=H, c=65),
                        in1=w.rearrange("p (t h) -> p t h", h=H)[:, :, :, None]
                            .to_broadcast([P, BT, H, 65]),
                        op=AluOp.mult)
                    if dbg:
                        for j in range(BT):
                            t = b0 + j
                            nc.sync.dma_start(
                                dbg_w[t, :, :],
                                w.rearrange("p (t h) -> p t h", h=H)[:, j, :])
                            nc.sync.dma_start(
                                dbg_dl[t, :, :],
                                dl.rearrange("p (t h) -> p t h", h=H)[:, j, :])
                            nc.sync.dma_start(dbg_oh[t, :, :],
                                              oh[:, j * P:(j + 1) * P])
                    for j in range(BT):
                        t = b0 + j
                        for h in range(H):
                            # one start=True per PSUM bank (= zero region):
                            # it arms the whole bank; later writes overwrite
                            # pending-zero bytes once, then accumulate.
                            cps = cps_a if h < 4 else cps_b
                            nc.tensor.matmul(
                                out=cps[:, (h % 4) * 65:(h % 4 + 1) * 65],
                                lhsT=oh[:, j * P: j * P + qn],
                                rhs=vgw[:, j * VW + h * 65: j * VW + (h + 1) * 65],
                                start=(t == t_lo and h % 4 == 0),
                                stop=(t == t_hi - 1 and h % 4 == 3),
                                skip_group_check=True)
                corr_sb = corr0 if qb == 0 else corr1
                nc.vector.tensor_copy(corr_sb[:qn, 0:4 * 65], cps_a[:])
                nc.vector.tensor_copy(corr_sb[:qn, 4 * 65:VW], cps_b[:])
                if dbg:
                    nc.sync.dma_start(dbg_corr[qb, 0:qn, :], corr_sb[:qn, :])

        # ================= phase 3: dense attention ==================
        with tc.tile_pool(name="pp", bufs=4) as pp, \
             tc.tile_pool(name="pt0", bufs=3, space="PSUM") as pt0p, \
             tc.tile_pool(name="avp", bufs=2, space="PSUM") as avp, \
             tc.tile_pool(name="ep", bufs=2) as ep:

            for h in range(H):
                avps = avp.tile([65, QC], FP, space="PSUM", tag="av")
                for rc in range(NB):
                    ps0 = pt0p.tile([P, QC], FP, space="PSUM", tag="pt0")
                    nc.tensor.matmul(
                        out=ps0[:],
                        lhsT=khatT[:, h * N + rc * P: h * N + (rc + 1) * P],
                        rhs=qhatT[:, h * QC:(h + 1) * QC],
                        start=True, stop=True)
                    p_sb = pp.tile([P, QC], FP, tag="p")
                    nc.scalar.activation(out=p_sb[:], in_=ps0[:], func=ActFn.Exp)
                    nc.tensor.matmul(
                        out=avps[:],
                        lhsT=vhat[:, rc * VW + h * 65: rc * VW + (h + 1) * 65],
                        rhs=p_sb[:],
                        start=(rc == 0), stop=False, skip_group_check=True)
                for qb, (qo, qn) in enumerate(QB):
                    corr_sb = corr0 if qb == 0 else corr1
                    nc.tensor.matmul(
                        out=avps[:, qo:qo + qn],
                        lhsT=corr_sb[:qn, h * 65:(h + 1) * 65],
                        rhs=ident[:qn, :qn],
                        start=False, stop=True, skip_group_check=True)
                if dbg:
                    av_sb = ep.tile([65, QC], FP, tag="avsb")
                    nc.vector.tensor_copy(av_sb[:], avps[:])
                    nc.sync.dma_start(dbg_av[h, :, :], av_sb[:])
                rec = ep.tile([1, QC], FP, tag="rec")
                nc.vector.reciprocal(out=rec[:], in_=avps[64:65, :])
                nc.sync.dma_start(rec_d[h:h + 1, :], rec[:])
                rec_rep = ep.tile([64, QC], FP, tag="recrep")
                nc.sync.dma_start(rec_rep[:],
                                  rec_d[h:h + 1, :].to_broadcast([64, QC]))
                onrm = ep.tile([64, QC], FP, tag="onrm")
                nc.vector.tensor_tensor(
                    out=onrm[:], in0=avps[0:64, :], in1=rec_rep[:],
                    op=AluOp.mult)
                mx = ep.tile([64, 1], FP, tag="mx")
                nc.vector.reduce_max(out=mx[:], in_=onrm[:],
                                     axis=mybir.AxisListType.X)
                nc.sync.dma_start(out_d[h * 64:(h + 1) * 64][:, None], mx[:])

    nc.compile()
    return nc


# ----------------------------------------------------------------- entry
def kernel(features, edge_index, edge_attr, pos, Wq, bq, Wk, bk, Wv, bv,
           _trace=False):
    features = np.asarray(features, np.float32)
    Wq = np.asarray(Wq, np.float32)
    Wk = np.asarray(Wk, np.float32)
    Wv = np.asarray(Wv, np.float32)
    for b in (bq, bk, bv):
        assert not np.any(np.asarray(b)), "nonzero biases unsupported"

    qe, re_, pev = _prep_edges(edge_index, edge_attr)
    T0, T1, emaps = _shard_edges(qe, re_, pev)

    featT = np.ascontiguousarray(features.T)
    wqT = np.ascontiguousarray(Wq.T)
    wkT = np.ascontiguousarray(Wk.T)
    wvT = np.ascontiguousarray(Wv.T)
    modd = np.zeros((D, H), np.float32)
    for h in range(H):
        modd[h * DH + 1: (h + 1) * DH: 2, h] = 1.0

    in_maps = []
    for c in range(NC):
        m = dict(featT=featT,
                 qfeatT=np.ascontiguousarray(featT[:, c * QC:(c + 1) * QC]),
                 wqT=wqT, wkT=wkT, wvT=wvT, modd=modd,
                 qli=emaps[c]["qli"], rei=emaps[c]["rei"],
                 qlf=emaps[c]["qlf"], pev=emaps[c]["pev"])
        in_maps.append(m)

    nc = _build(T0, T1)
    res = run_bass_kernel_spmd(nc, in_maps, list(range(NC)), trace=_trace)
    parts = np.stack([res.results[c]["out_part"] for c in range(NC)])
    out = parts.max(axis=0)
    if _trace:
        return out, res
    return out
